# revision 43
# baseline (speedup 1.0000x reference)
"""CAM (channel self-attention) kernel for Trainium2 — 8 NeuronCores, batch-parallel.

Math per batch element b (A = x[b] reshaped [N=4096, C=512]):
    G = A^T A                  [C, C]   (symmetric!)
    P = softmax_rows(G)        [C, C]
    Y = A P                    [N, C]
    out = gamma * Y + x

Sharding: data-parallel over batch — core i handles batch element i.
No cross-core communication needed.

Per-core schedule (fp8 DoubleRow pipeline):
  - DMA x in 1 MiB groups -> A32 (f32, resident); DVE cast to A8 (fp8e4).
  - Per 128-row chunk k: 4 PE transposes of the A8 chunk (fp8 -> PSUM) and
    one strided ACT copy into AT8; per chunk PAIR, 4 upper-triangle Gram
    matmuls in fp8 DoubleRow mode (contraction 256 rows/MM, free dims
    512/384/256/128 exploiting G's symmetry).  PE work sits below the DMA
    cadence, so phase 1 is load-bound.
  - Lower triangle of G via 6 f32 PE transposes of the upper blocks.
  - softmax: DVE row-max (negated) -> ACT exp with fused row-sum -> DVE
    reciprocal -> one dual-scalar DVE op P8 = E * rsum * gamma.  gamma is
    folded into P here, so the epilogue is a pure add and gamma==0 gives
    bit-exact out == x regardless of fp8 rounding in Y.
  - Y' = A (gamma*P) via 2 fp8 DoubleRow matmuls per 128-row chunk
    (lhsT = AT8 pair-slice, rhs = P8 pair-slice).
  - epilogue: out = Y' + A32, alternating DVE / GpSimd scalar_tensor_tensor,
    staged in 512 KiB groups, DMA'd out (last groups on the ACT ring).
"""

import numpy as np

import concourse.tile as tile
from concourse import bacc, mybir
from concourse.bass_utils import run_bass_kernel_spmd
from concourse.masks import make_identity

B = 8
H = 64
W = 64
C = 512
HW = H * W            # 4096 rows per batch element
NT = HW // 128        # 32 row chunks of 128
CT = C // 128         # 4 col chunks of 128
GRP = 4               # row chunks per input DMA group (1 MiB)
OGRP = 2              # row chunks per output DMA group (512 KiB)
ONG = NT // OGRP      # 16 output groups
WARM_MMS = 5          # N=512 warmup matmuls (fills the pre-first-chunk gap)
AT_DEFER = 4          # trailing chunks whose A^T build is deferred past softmax

F32 = mybir.dt.float32
BF16 = mybir.dt.bfloat16
FP8 = mybir.dt.float8e4

_CACHE = {}


def _emit(nc, tc, out, x, gamma):
    from contextlib import ExitStack

    DR = mybir.MatmulPerfMode.DoubleRow

    with ExitStack() as ctx:
        big = ctx.enter_context(tc.tile_pool(name="big", bufs=1))
        small = ctx.enter_context(tc.tile_pool(name="small", bufs=1))
        stat = ctx.enter_context(tc.tile_pool(name="stat", bufs=4))
        ostage = ctx.enter_context(tc.tile_pool(name="ostage", bufs=4))
        gps = ctx.enter_context(tc.tile_pool(name="gps", bufs=1, space="PSUM"))
        wps = ctx.enter_context(tc.tile_pool(name="wps", bufs=5, space="PSUM"))

        A32 = big.tile([128, NT, C], F32)     # x rows, n on partitions
        A8 = big.tile([128, NT, C], FP8)      # fp8 cast of A32
        AT8 = big.tile([128, CT, HW], FP8)    # A^T, c on partitions
        G32 = big.tile([128, CT, C], F32)     # full Gram matrix in SBUF
        E32 = big.tile([128, CT, C], F32)     # exp(G - rowmax)
        P8 = big.tile([128, CT, C], FP8)      # gamma * softmax(G) in fp8

        ident8 = small.tile([128, 128], FP8)
        make_identity(nc, ident8[:])
        ident32 = small.tile([128, 128], F32)
        make_identity(nc, ident32[:])

        gB = small.tile([128, 1], F32)        # gamma broadcast to all partitions

        # PE warm-up: the HAM clock gate holds the PE at 1.2 GHz until it has
        # been ~3.4us busy.  Burn the dead window before the first input chunk
        # lands with a few dummy matmuls; sized so real work isn't delayed.
        warm_src = small.tile([128, C], BF16)
        nc.gpsimd.memset(warm_src[:], 0.0)
        warm_ps = wps.tile([128, C], F32, name="warm", tag="w")
        for wi in range(WARM_MMS):
            nc.tensor.matmul(
                warm_ps[:], warm_src[:, 0:128], warm_src[:],
                start=(wi == 0), stop=(wi == WARM_MMS - 1),
            )

        # Upper-triangle Gram accumulators: G[mi-chunk, mi*128:].
        # g1 (384 cols) and g3 (128 cols) share one PSUM bank.
        g0 = gps.tile([128, C], F32, name="g0", tag="g0")
        g13 = gps.tile([128, C], F32, name="g13", tag="g13")
        g2 = gps.tile([128, C - 256], F32, name="g2", tag="g2")
        g_ps = [g0[:], g13[:, 0:384], g2[:], g13[:, 384:512]]

        # First loads chunk-granular so the PE can start early, then 1 MiB.
        load_groups = [1, 1, 2] + [GRP] * ((NT - 4) // GRP)
        k0 = 0
        for gi, gsz in enumerate(load_groups):
            r0 = k0 * 128
            r1 = (k0 + gsz) * 128
            nc.sync.dma_start(
                A32[:, k0:k0 + gsz, :],
                x[r0:r1, :].rearrange("(t p) c -> p t c", p=128),
            )
            if gi == 0:
                # gamma: tiny load on the ACT HWDGE ring, off the input path
                nc.scalar.dma_start(gB[:], gamma[:])
            for j in range(gsz):
                k = k0 + j
                # cast f32 -> fp8 (DVE; keeps ACT free for A^T copies)
                nc.vector.tensor_copy(A8[:, k, :], A32[:, k, :])
                # A^T blocks of this chunk -> one PSUM bank, one strided copy.
                # fp8 PE transposes write PSUM at 2-byte pitch, so the tile
                # carries an explicit pitch dim and all views use step 2.
                # The LAST chunks' A^T is deferred past the softmax emission:
                # their Y consumers run at the very end of phase 2, and at
                # load-end the G-copy -> softmax chain would otherwise queue
                # on ACT behind these copies.  (Gram below still runs for
                # every chunk.)
                if k < NT - AT_DEFER:
                    tp = wps.tile([128, CT * 128, 2], FP8, name="tp", tag="w")
                    for ci in range(CT):
                        nc.tensor.transpose(
                            tp[:, ci * 128:(ci + 1) * 128, 0],
                            A8[:, k, ci * 128:(ci + 1) * 128],
                            ident8[:],
                        )
                    nc.scalar.copy(
                        AT8[:, :, k * 128:(k + 1) * 128],
                        tp[:, :, 0].rearrange("p (ci n) -> p ci n", ci=CT),
                    )
                # upper-triangle Gram matmuls, fp8 DoubleRow on chunk pairs
                if k % 2 == 1:
                    for mi in range(CT):
                        nc.tensor.matmul(
                            g_ps[mi],
                            A8[:, k - 1:k + 1, mi * 128:(mi + 1) * 128],
                            A8[:, k - 1:k + 1, mi * 128:],
                            start=(k == 1),
                            stop=(k == NT - 1),
                            perf_mode=DR,
                            # g1/g3 share a bank; per-element has_written makes
                            # disjoint-region groups safe on HW
                            skip_group_check=(mi % 2 == 1),
                        )
            k0 += gsz

        # G (upper) PSUM -> SBUF
        for mi in range(CT):
            if mi % 2 == 0:
                nc.vector.tensor_copy(G32[:, mi, mi * 128:], g_ps[mi])
            else:
                nc.scalar.copy(G32[:, mi, mi * 128:], g_ps[mi])
        # reconstruct lower triangle: G[mi, j] = G[j, mi]^T for j < mi
        for mi in range(1, CT):
            for j in range(mi):
                lb = wps.tile([128, 128], F32, name="lb", tag="w")
                nc.tensor.transpose(
                    lb[:], G32[:, j, mi * 128:(mi + 1) * 128], ident32[:])
                if (mi + j) % 2 == 0:
                    nc.vector.tensor_copy(G32[:, mi, j * 128:(j + 1) * 128], lb[:])
                else:
                    nc.scalar.copy(G32[:, mi, j * 128:(j + 1) * 128], lb[:])

        # softmax over rows of G (free axis); gamma folded into the scale
        for mi in range(CT):
            nmax = stat.tile([128, 1], F32)
            nc.vector.tensor_reduce(
                nmax[:], G32[:, mi, :],
                axis=mybir.AxisListType.X, op=mybir.AluOpType.max, negate=True,
            )
            esum = stat.tile([128, 1], F32)
            nc.scalar.activation(
                E32[:, mi, :], G32[:, mi, :],
                mybir.ActivationFunctionType.Exp,
                bias=nmax[:], scale=1.0, accum_out=esum[:],
            )
            rsum = stat.tile([128, 1], F32)
            nc.vector.reciprocal(rsum[:], esum[:])
            # P8 = (E * (1/rowsum)) * gamma  — one dual-scalar DVE op
            nc.vector.tensor_scalar(
                P8[:, mi, :], E32[:, mi, :], rsum[:], gB[:],
                op0=mybir.AluOpType.mult, op1=mybir.AluOpType.mult,
            )

        # deferred A^T for the last chunks (ACT and PE both have slack here)
        for k in range(NT - AT_DEFER, NT):
            tp = wps.tile([128, CT * 128, 2], FP8, name="tp", tag="w")
            for ci in range(CT):
                nc.tensor.transpose(
                    tp[:, ci * 128:(ci + 1) * 128, 0],
                    A8[:, k, ci * 128:(ci + 1) * 128],
                    ident8[:],
                )
            nc.scalar.copy(
                AT8[:, :, k * 128:(k + 1) * 128],
                tp[:, :, 0].rearrange("p (ci n) -> p ci n", ci=CT),
            )

        # Y' = A @ (gamma P) via fp8 DoubleRow; epilogue out = Y' + x.
        out_groups = [OGRP] * (ONG - 1) + [1, 1]
        assert sum(out_groups) == NT
        t0 = 0
        for h, osz in enumerate(out_groups):
            r0 = t0 * 128
            r1 = (t0 + osz) * 128
            o32 = ostage.tile([128, OGRP, C], F32)
            for j in range(osz):
                t = t0 + j
                y = wps.tile([128, C], F32, name="y", tag="w")
                for cp in range(0, CT, 2):
                    nc.tensor.matmul(
                        y[:],
                        AT8[:, cp:cp + 2, t * 128:(t + 1) * 128],
                        P8[:, cp:cp + 2, :],
                        start=(cp == 0),
                        stop=(cp == CT - 2),
                        perf_mode=DR,
                    )
                nc.vector.scalar_tensor_tensor(
                    o32[:, j, :], y[:], 1.0, A32[:, t, :],
                    op0=mybir.AluOpType.mult, op1=mybir.AluOpType.add,
                )
            # last groups ride the idle ACT ring to dodge Sync-ring backlog
            oeng = nc.scalar if h >= len(out_groups) - 2 else nc.sync
            oeng.dma_start(
                out[r0:r1, :].rearrange("(t p) c -> p t c", p=128),
                o32[:, 0:osz, :],
            )
            t0 += osz


def build(compile_=True):
    nc = bacc.Bacc("TRN2", target_bir_lowering=False, debug=False)
    x = nc.dram_tensor("x", [HW, C], F32, kind="ExternalInput").ap()
    gamma = nc.dram_tensor("gamma", [128, 1], F32, kind="ExternalInput").ap()
    out = nc.dram_tensor("out", [HW, C], F32, kind="ExternalOutput").ap()
    with tile.TileContext(nc) as tc:
        _emit(nc, tc, out, x, gamma)
    if compile_:
        nc.compile()
    return nc


def kernel(x: np.ndarray, gamma: np.ndarray, trace: bool = False):
    assert x.shape == (B, H, W, C), x.shape
    if "nc" not in _CACHE:
        _CACHE["nc"] = build()
    nc = _CACHE["nc"]

    g128 = np.full((128, 1), np.float32(np.asarray(gamma).reshape(-1)[0]),
                   dtype=np.float32)
    in_maps = [
        {
            "x": np.ascontiguousarray(
                np.asarray(x[i], dtype=np.float32).reshape(HW, C)),
            "gamma": g128,
        }
        for i in range(B)
    ]
    if trace:
        res = run_bass_kernel_spmd(nc, in_maps, core_ids=list(range(B)),
                                   trace=True)
    else:
        # Force-untraced: a stray BASS_TRACE in the environment would route
        # through profiling hooks this image may not have.
        import os
        prev = os.environ.get("BASS_NEVER_TRACE")
        os.environ["BASS_NEVER_TRACE"] = "1"
        try:
            res = run_bass_kernel_spmd(nc, in_maps, core_ids=list(range(B)))
        finally:
            if prev is None:
                os.environ.pop("BASS_NEVER_TRACE", None)
            else:
                os.environ["BASS_NEVER_TRACE"] = prev
    _CACHE["last_result"] = res
    out = np.stack([res.results[i]["out"] for i in range(B)], axis=0)
    return out.reshape(B, H, W, C).astype(np.float32)


# revision 44
# speedup vs baseline: 1.0400x; 1.0400x over previous
"""CAM (channel self-attention) kernel for Trainium2 — 8 NeuronCores, batch-parallel.

Math per batch element b (A = x[b] reshaped [N=4096, C=512]):
    G = A^T A                  [C, C]   (symmetric!)
    P = softmax_rows(G)        [C, C]
    Y = A P                    [N, C]
    out = gamma * Y + x

Sharding: data-parallel over batch — core i handles batch element i.
No cross-core communication needed.

Per-core schedule (fp8 DoubleRow pipeline):
  - DMA x in 1 MiB groups -> A32 (f32, resident); DVE cast to A8 (fp8e4).
  - Per 128-row chunk k: 4 PE transposes of the A8 chunk (fp8 -> PSUM) and
    one strided ACT copy into AT8; per chunk PAIR, 4 upper-triangle Gram
    matmuls in fp8 DoubleRow mode (contraction 256 rows/MM, free dims
    512/384/256/128 exploiting G's symmetry).  PE work sits below the DMA
    cadence, so phase 1 is load-bound.
  - Lower triangle of G via 6 f32 PE transposes of the upper blocks.
  - softmax: DVE row-max (negated) -> ACT exp with fused row-sum -> DVE
    reciprocal -> one dual-scalar DVE op P8 = E * rsum * gamma.  gamma is
    folded into P here, so the epilogue is a pure add and gamma==0 gives
    bit-exact out == x regardless of fp8 rounding in Y.
  - Y' = A (gamma*P) via 2 fp8 DoubleRow matmuls per 128-row chunk
    (lhsT = AT8 pair-slice, rhs = P8 pair-slice).
  - epilogue: out = Y' + A32, alternating DVE / GpSimd scalar_tensor_tensor,
    staged in 512 KiB groups, DMA'd out (last groups on the ACT ring).
"""

import numpy as np

import concourse.tile as tile
from concourse import bacc, mybir
from concourse.bass_utils import run_bass_kernel_spmd
from concourse.masks import make_identity

B = 8
H = 64
W = 64
C = 512
HW = H * W            # 4096 rows per batch element
NT = HW // 128        # 32 row chunks of 128
CT = C // 128         # 4 col chunks of 128
GRP = 4               # row chunks per input DMA group (1 MiB)
OGRP = 2              # row chunks per output DMA group (512 KiB)
ONG = NT // OGRP      # 16 output groups
WARM_MMS = 5          # N=512 warmup matmuls (fills the pre-first-chunk gap)
AT_DEFER = 4          # trailing chunks whose A^T build is deferred past softmax

F32 = mybir.dt.float32
BF16 = mybir.dt.bfloat16
FP8 = mybir.dt.float8e4

_CACHE = {}


def _emit(nc, tc, out, x, gamma):
    from contextlib import ExitStack

    DR = mybir.MatmulPerfMode.DoubleRow

    with ExitStack() as ctx:
        big = ctx.enter_context(tc.tile_pool(name="big", bufs=1))
        small = ctx.enter_context(tc.tile_pool(name="small", bufs=1))
        stat = ctx.enter_context(tc.tile_pool(name="stat", bufs=4))
        ostage = ctx.enter_context(tc.tile_pool(name="ostage", bufs=4))
        gps = ctx.enter_context(tc.tile_pool(name="gps", bufs=1, space="PSUM"))
        wps = ctx.enter_context(tc.tile_pool(name="wps", bufs=5, space="PSUM"))

        A32 = big.tile([128, NT, C], F32)     # x rows, n on partitions
        A8 = big.tile([128, NT, C], FP8)      # fp8 cast of A32
        AT8 = big.tile([128, CT, HW], FP8)    # A^T, c on partitions
        G32 = big.tile([128, CT, C], F32)     # full Gram matrix in SBUF
        E32 = big.tile([128, CT, C], F32)     # exp(G - rowmax)
        P8 = big.tile([128, CT, C], FP8)      # gamma * softmax(G) in fp8

        ident8 = small.tile([128, 128], FP8)
        make_identity(nc, ident8[:])
        ident32 = small.tile([128, 128], F32)
        make_identity(nc, ident32[:])

        gB = small.tile([128, 1], F32)        # gamma broadcast to all partitions

        # PE warm-up: the HAM clock gate holds the PE at 1.2 GHz until it has
        # been ~3.4us busy.  Burn the dead window before the first input chunk
        # lands with a few dummy matmuls; sized so real work isn't delayed.
        warm_src = small.tile([128, C], BF16)
        nc.gpsimd.memset(warm_src[:], 0.0)
        warm_ps = wps.tile([128, C], F32, name="warm", tag="w")
        for wi in range(WARM_MMS):
            nc.tensor.matmul(
                warm_ps[:], warm_src[:, 0:128], warm_src[:],
                start=(wi == 0), stop=(wi == WARM_MMS - 1),
            )

        # Upper-triangle Gram accumulators: G[mi-chunk, mi*128:].
        # g1 (384 cols) and g3 (128 cols) share one PSUM bank.
        g0 = gps.tile([128, C], F32, name="g0", tag="g0")
        g13 = gps.tile([128, C], F32, name="g13", tag="g13")
        g2 = gps.tile([128, C - 256], F32, name="g2", tag="g2")
        g_ps = [g0[:], g13[:, 0:384], g2[:], g13[:, 384:512]]

        # First loads chunk-granular so the PE can start early, then 1 MiB.
        load_groups = [1, 1, 2] + [GRP] * ((NT - 4) // GRP)
        k0 = 0
        for gi, gsz in enumerate(load_groups):
            r0 = k0 * 128
            r1 = (k0 + gsz) * 128
            nc.sync.dma_start(
                A32[:, k0:k0 + gsz, :],
                x[r0:r1, :].rearrange("(t p) c -> p t c", p=128),
            )
            if gi == 0:
                # gamma: tiny load on the ACT HWDGE ring, off the input path
                nc.scalar.dma_start(gB[:], gamma[:])
            for j in range(gsz):
                k = k0 + j
                # cast f32 -> fp8 (DVE; keeps ACT free for A^T copies)
                nc.vector.tensor_copy(A8[:, k, :], A32[:, k, :])
                # A^T blocks of this chunk -> one PSUM bank, one strided copy.
                # fp8 PE transposes write PSUM at 2-byte pitch, so the tile
                # carries an explicit pitch dim and all views use step 2.
                # The LAST chunks' A^T is deferred past the softmax emission:
                # their Y consumers run at the very end of phase 2, and at
                # load-end the G-copy -> softmax chain would otherwise queue
                # on ACT behind these copies.  (Gram below still runs for
                # every chunk.)
                if k < NT - AT_DEFER:
                    tp = wps.tile([128, CT * 128, 2], FP8, name="tp", tag="w")
                    for ci in range(CT):
                        nc.tensor.transpose(
                            tp[:, ci * 128:(ci + 1) * 128, 0],
                            A8[:, k, ci * 128:(ci + 1) * 128],
                            ident8[:],
                        )
                    nc.scalar.copy(
                        AT8[:, :, k * 128:(k + 1) * 128],
                        tp[:, :, 0].rearrange("p (ci n) -> p ci n", ci=CT),
                    )
                # upper-triangle Gram matmuls, fp8 DoubleRow on chunk pairs
                if k % 2 == 1:
                    for mi in range(CT):
                        nc.tensor.matmul(
                            g_ps[mi],
                            A8[:, k - 1:k + 1, mi * 128:(mi + 1) * 128],
                            A8[:, k - 1:k + 1, mi * 128:],
                            start=(k == 1),
                            stop=(k == NT - 1),
                            perf_mode=DR,
                            # g1/g3 share a bank; per-element has_written makes
                            # disjoint-region groups safe on HW
                            skip_group_check=(mi % 2 == 1),
                        )
            k0 += gsz

        # G (upper) PSUM -> SBUF
        for mi in range(CT):
            if mi % 2 == 0:
                nc.vector.tensor_copy(G32[:, mi, mi * 128:], g_ps[mi])
            else:
                nc.scalar.copy(G32[:, mi, mi * 128:], g_ps[mi])
        # reconstruct lower triangle: G[mi, j] = G[j, mi]^T for j < mi
        for mi in range(1, CT):
            for j in range(mi):
                lb = wps.tile([128, 128], F32, name="lb", tag="w")
                nc.tensor.transpose(
                    lb[:], G32[:, j, mi * 128:(mi + 1) * 128], ident32[:])
                if (mi + j) % 2 == 0:
                    nc.vector.tensor_copy(G32[:, mi, j * 128:(j + 1) * 128], lb[:])
                else:
                    nc.scalar.copy(G32[:, mi, j * 128:(j + 1) * 128], lb[:])

        # softmax over rows of G (free axis); gamma folded into the scale
        for mi in range(CT):
            nmax = stat.tile([128, 1], F32)
            nc.vector.tensor_reduce(
                nmax[:], G32[:, mi, :],
                axis=mybir.AxisListType.X, op=mybir.AluOpType.max, negate=True,
            )
            esum = stat.tile([128, 1], F32)
            nc.scalar.activation(
                E32[:, mi, :], G32[:, mi, :],
                mybir.ActivationFunctionType.Exp,
                bias=nmax[:], scale=1.0, accum_out=esum[:],
            )
            rsum = stat.tile([128, 1], F32)
            nc.vector.reciprocal(rsum[:], esum[:])
            # P8 = (E * (1/rowsum)) * gamma  — one dual-scalar DVE op
            nc.vector.tensor_scalar(
                P8[:, mi, :], E32[:, mi, :], rsum[:], gB[:],
                op0=mybir.AluOpType.mult, op1=mybir.AluOpType.mult,
            )

        # Y' = A @ (gamma P) via fp8 DoubleRow; epilogue out = Y' + x.
        # The deferred A^T builds are interleaved at later group boundaries:
        # emitted after the early Y matmuls (so they never delay the Y ramp)
        # but well before their own consumers (Y chunks NT-AT_DEFER..NT-1).
        def emit_deferred_at(k):
            tp = wps.tile([128, CT * 128, 2], FP8, name="tp", tag="w")
            for ci in range(CT):
                nc.tensor.transpose(
                    tp[:, ci * 128:(ci + 1) * 128, 0],
                    A8[:, k, ci * 128:(ci + 1) * 128],
                    ident8[:],
                )
            nc.scalar.copy(
                AT8[:, :, k * 128:(k + 1) * 128],
                tp[:, :, 0].rearrange("p (ci n) -> p ci n", ci=CT),
            )

        out_groups = [OGRP] * (ONG - 1) + [1, 1]
        assert sum(out_groups) == NT
        t0 = 0
        for h, osz in enumerate(out_groups):
            if h in (3, 4, 5, 6):
                emit_deferred_at(NT - AT_DEFER + (h - 3))
            r0 = t0 * 128
            r1 = (t0 + osz) * 128
            o32 = ostage.tile([128, OGRP, C], F32)
            for j in range(osz):
                t = t0 + j
                y = wps.tile([128, C], F32, name="y", tag="w")
                for cp in range(0, CT, 2):
                    nc.tensor.matmul(
                        y[:],
                        AT8[:, cp:cp + 2, t * 128:(t + 1) * 128],
                        P8[:, cp:cp + 2, :],
                        start=(cp == 0),
                        stop=(cp == CT - 2),
                        perf_mode=DR,
                    )
                nc.vector.scalar_tensor_tensor(
                    o32[:, j, :], y[:], 1.0, A32[:, t, :],
                    op0=mybir.AluOpType.mult, op1=mybir.AluOpType.add,
                )
            # last groups ride the idle ACT ring to dodge Sync-ring backlog
            oeng = nc.scalar if h >= len(out_groups) - 2 else nc.sync
            oeng.dma_start(
                out[r0:r1, :].rearrange("(t p) c -> p t c", p=128),
                o32[:, 0:osz, :],
            )
            t0 += osz


def build(compile_=True):
    nc = bacc.Bacc("TRN2", target_bir_lowering=False, debug=False)
    x = nc.dram_tensor("x", [HW, C], F32, kind="ExternalInput").ap()
    gamma = nc.dram_tensor("gamma", [128, 1], F32, kind="ExternalInput").ap()
    out = nc.dram_tensor("out", [HW, C], F32, kind="ExternalOutput").ap()
    with tile.TileContext(nc) as tc:
        _emit(nc, tc, out, x, gamma)
    if compile_:
        nc.compile()
    return nc


def kernel(x: np.ndarray, gamma: np.ndarray, trace: bool = False):
    assert x.shape == (B, H, W, C), x.shape
    if "nc" not in _CACHE:
        _CACHE["nc"] = build()
    nc = _CACHE["nc"]

    g128 = np.full((128, 1), np.float32(np.asarray(gamma).reshape(-1)[0]),
                   dtype=np.float32)
    in_maps = [
        {
            "x": np.ascontiguousarray(
                np.asarray(x[i], dtype=np.float32).reshape(HW, C)),
            "gamma": g128,
        }
        for i in range(B)
    ]
    if trace:
        res = run_bass_kernel_spmd(nc, in_maps, core_ids=list(range(B)),
                                   trace=True)
    else:
        # Force-untraced: a stray BASS_TRACE in the environment would route
        # through profiling hooks this image may not have.
        import os
        prev = os.environ.get("BASS_NEVER_TRACE")
        os.environ["BASS_NEVER_TRACE"] = "1"
        try:
            res = run_bass_kernel_spmd(nc, in_maps, core_ids=list(range(B)))
        finally:
            if prev is None:
                os.environ.pop("BASS_NEVER_TRACE", None)
            else:
                os.environ["BASS_NEVER_TRACE"] = prev
    _CACHE["last_result"] = res
    out = np.stack([res.results[i]["out"] for i in range(B)], axis=0)
    return out.reshape(B, H, W, C).astype(np.float32)


# revision 45
# speedup vs baseline: 1.0451x; 1.0049x over previous
"""CAM (channel self-attention) kernel for Trainium2 — 8 NeuronCores, batch-parallel.

Math per batch element b (A = x[b] reshaped [N=4096, C=512]):
    G = A^T A                  [C, C]   (symmetric!)
    P = softmax_rows(G)        [C, C]
    Y = A P                    [N, C]
    out = gamma * Y + x

Sharding: data-parallel over batch — core i handles batch element i.
No cross-core communication needed.

Per-core schedule (fp8 DoubleRow pipeline):
  - DMA x in 1 MiB groups -> A32 (f32, resident); DVE cast to A8 (fp8e4).
  - Per 128-row chunk k: 4 PE transposes of the A8 chunk (fp8 -> PSUM) and
    one strided ACT copy into AT8; per chunk PAIR, 4 upper-triangle Gram
    matmuls in fp8 DoubleRow mode (contraction 256 rows/MM, free dims
    512/384/256/128 exploiting G's symmetry).  PE work sits below the DMA
    cadence, so phase 1 is load-bound.
  - Lower triangle of G via 6 f32 PE transposes of the upper blocks.
  - softmax: DVE row-max (negated) -> ACT exp with fused row-sum -> DVE
    reciprocal -> one dual-scalar DVE op P8 = E * rsum * gamma.  gamma is
    folded into P here, so the epilogue is a pure add and gamma==0 gives
    bit-exact out == x regardless of fp8 rounding in Y.
  - Y' = A (gamma*P) via 2 fp8 DoubleRow matmuls per 128-row chunk
    (lhsT = AT8 pair-slice, rhs = P8 pair-slice).
  - epilogue: out = Y' + A32 via DVE scalar_tensor_tensor, staged in
    512 KiB groups, DMA'd out (last groups on the ACT ring).
"""

import numpy as np

import concourse.tile as tile
from concourse import bacc, mybir
from concourse.bass_utils import run_bass_kernel_spmd
from concourse.masks import make_identity

B = 8
H = 64
W = 64
C = 512
HW = H * W            # 4096 rows per batch element
NT = HW // 128        # 32 row chunks of 128
CT = C // 128         # 4 col chunks of 128
GRP = 4               # row chunks per input DMA group (1 MiB)
OGRP = 2              # row chunks per output DMA group (512 KiB)
ONG = NT // OGRP      # 16 output groups
WARM_MMS = 5          # N=512 warmup matmuls (fills the pre-first-chunk gap)

F32 = mybir.dt.float32
BF16 = mybir.dt.bfloat16
FP8 = mybir.dt.float8e4

_CACHE = {}


def _emit(nc, tc, out, x, gamma):
    from contextlib import ExitStack

    DR = mybir.MatmulPerfMode.DoubleRow

    with ExitStack() as ctx:
        big = ctx.enter_context(tc.tile_pool(name="big", bufs=1))
        small = ctx.enter_context(tc.tile_pool(name="small", bufs=1))
        stat = ctx.enter_context(tc.tile_pool(name="stat", bufs=4))
        ostage = ctx.enter_context(tc.tile_pool(name="ostage", bufs=4))
        gps = ctx.enter_context(tc.tile_pool(name="gps", bufs=1, space="PSUM"))
        wps = ctx.enter_context(tc.tile_pool(name="wps", bufs=5, space="PSUM"))

        A32 = big.tile([128, NT, C], F32)     # x rows, n on partitions
        A8 = big.tile([128, NT, C], FP8)      # fp8 cast of A32
        AT8 = big.tile([128, CT, HW], FP8)    # A^T, c on partitions
        G32 = big.tile([128, CT, C], F32)     # full Gram matrix in SBUF
        E32 = big.tile([128, CT, C], F32)     # exp(G - rowmax)
        P8 = big.tile([128, CT, C], FP8)      # gamma * softmax(G) in fp8

        ident8 = small.tile([128, 128], FP8)
        make_identity(nc, ident8[:])
        ident32 = small.tile([128, 128], F32)
        make_identity(nc, ident32[:])

        gB = small.tile([128, 1], F32)        # gamma broadcast to all partitions

        # PE warm-up: the HAM clock gate holds the PE at 1.2 GHz until it has
        # been ~3.4us busy.  Burn the dead window before the first input chunk
        # lands with a few dummy matmuls; sized so real work isn't delayed.
        warm_src = small.tile([128, C], BF16)
        nc.gpsimd.memset(warm_src[:], 0.0)
        warm_ps = wps.tile([128, C], F32, name="warm", tag="w")
        for wi in range(WARM_MMS):
            nc.tensor.matmul(
                warm_ps[:], warm_src[:, 0:128], warm_src[:],
                start=(wi == 0), stop=(wi == WARM_MMS - 1),
            )

        # Upper-triangle Gram accumulators: G[mi-chunk, mi*128:].
        # g1 (384 cols) and g3 (128 cols) share one PSUM bank.
        g0 = gps.tile([128, C], F32, name="g0", tag="g0")
        g13 = gps.tile([128, C], F32, name="g13", tag="g13")
        g2 = gps.tile([128, C - 256], F32, name="g2", tag="g2")
        g_ps = [g0[:], g13[:, 0:384], g2[:], g13[:, 384:512]]

        # First loads chunk-granular so the PE can start early, then 1 MiB.
        load_groups = [1, 1, 2] + [GRP] * ((NT - 4) // GRP)
        k0 = 0
        for gi, gsz in enumerate(load_groups):
            r0 = k0 * 128
            r1 = (k0 + gsz) * 128
            nc.sync.dma_start(
                A32[:, k0:k0 + gsz, :],
                x[r0:r1, :].rearrange("(t p) c -> p t c", p=128),
            )
            if gi == 0:
                # gamma: tiny load on the ACT HWDGE ring, off the input path
                nc.scalar.dma_start(gB[:], gamma[:])
            for j in range(gsz):
                k = k0 + j
                # cast f32 -> fp8 (DVE; keeps ACT free for A^T copies)
                nc.vector.tensor_copy(A8[:, k, :], A32[:, k, :])
                # A^T blocks of this chunk -> one PSUM bank, one strided copy.
                # fp8 PE transposes write PSUM at 2-byte pitch, so the tile
                # carries an explicit pitch dim and all views use step 2.
                tp = wps.tile([128, CT * 128, 2], FP8, name="tp", tag="w")
                for ci in range(CT):
                    nc.tensor.transpose(
                        tp[:, ci * 128:(ci + 1) * 128, 0],
                        A8[:, k, ci * 128:(ci + 1) * 128],
                        ident8[:],
                    )
                nc.scalar.copy(
                    AT8[:, :, k * 128:(k + 1) * 128],
                    tp[:, :, 0].rearrange("p (ci n) -> p ci n", ci=CT),
                )
                # upper-triangle Gram matmuls, fp8 DoubleRow on chunk pairs
                if k % 2 == 1:
                    for mi in range(CT):
                        nc.tensor.matmul(
                            g_ps[mi],
                            A8[:, k - 1:k + 1, mi * 128:(mi + 1) * 128],
                            A8[:, k - 1:k + 1, mi * 128:],
                            start=(k == 1),
                            stop=(k == NT - 1),
                            perf_mode=DR,
                            # g1/g3 share a bank; per-element has_written makes
                            # disjoint-region groups safe on HW
                            skip_group_check=(mi % 2 == 1),
                        )
            k0 += gsz

        # G (upper) PSUM -> SBUF
        for mi in range(CT):
            if mi % 2 == 0:
                nc.vector.tensor_copy(G32[:, mi, mi * 128:], g_ps[mi])
            else:
                nc.scalar.copy(G32[:, mi, mi * 128:], g_ps[mi])
        # reconstruct lower triangle: G[mi, j] = G[j, mi]^T for j < mi
        for mi in range(1, CT):
            for j in range(mi):
                lb = wps.tile([128, 128], F32, name="lb", tag="w")
                nc.tensor.transpose(
                    lb[:], G32[:, j, mi * 128:(mi + 1) * 128], ident32[:])
                if (mi + j) % 2 == 0:
                    nc.vector.tensor_copy(G32[:, mi, j * 128:(j + 1) * 128], lb[:])
                else:
                    nc.scalar.copy(G32[:, mi, j * 128:(j + 1) * 128], lb[:])

        # softmax over rows of G (free axis); gamma folded into the scale
        for mi in range(CT):
            nmax = stat.tile([128, 1], F32)
            nc.vector.tensor_reduce(
                nmax[:], G32[:, mi, :],
                axis=mybir.AxisListType.X, op=mybir.AluOpType.max, negate=True,
            )
            esum = stat.tile([128, 1], F32)
            nc.scalar.activation(
                E32[:, mi, :], G32[:, mi, :],
                mybir.ActivationFunctionType.Exp,
                bias=nmax[:], scale=1.0, accum_out=esum[:],
            )
            rsum = stat.tile([128, 1], F32)
            nc.vector.reciprocal(rsum[:], esum[:])
            # P8 = (E * (1/rowsum)) * gamma  — one dual-scalar DVE op
            nc.vector.tensor_scalar(
                P8[:, mi, :], E32[:, mi, :], rsum[:], gB[:],
                op0=mybir.AluOpType.mult, op1=mybir.AluOpType.mult,
            )

        # Y' = A @ (gamma P) via fp8 DoubleRow; epilogue out = Y' + x.
        out_groups = [OGRP] * (ONG - 1) + [1, 1]
        assert sum(out_groups) == NT
        t0 = 0
        for h, osz in enumerate(out_groups):
            r0 = t0 * 128
            r1 = (t0 + osz) * 128
            o32 = ostage.tile([128, OGRP, C], F32)
            for j in range(osz):
                t = t0 + j
                y = wps.tile([128, C], F32, name="y", tag="w")
                for cp in range(0, CT, 2):
                    nc.tensor.matmul(
                        y[:],
                        AT8[:, cp:cp + 2, t * 128:(t + 1) * 128],
                        P8[:, cp:cp + 2, :],
                        start=(cp == 0),
                        stop=(cp == CT - 2),
                        perf_mode=DR,
                    )
                nc.vector.scalar_tensor_tensor(
                    o32[:, j, :], y[:], 1.0, A32[:, t, :],
                    op0=mybir.AluOpType.mult, op1=mybir.AluOpType.add,
                )
            # last groups ride the idle ACT ring to dodge Sync-ring backlog
            oeng = nc.scalar if h >= len(out_groups) - 2 else nc.sync
            oeng.dma_start(
                out[r0:r1, :].rearrange("(t p) c -> p t c", p=128),
                o32[:, 0:osz, :],
            )
            t0 += osz


def build(compile_=True):
    nc = bacc.Bacc("TRN2", target_bir_lowering=False, debug=False)
    x = nc.dram_tensor("x", [HW, C], F32, kind="ExternalInput").ap()
    gamma = nc.dram_tensor("gamma", [128, 1], F32, kind="ExternalInput").ap()
    out = nc.dram_tensor("out", [HW, C], F32, kind="ExternalOutput").ap()
    with tile.TileContext(nc) as tc:
        _emit(nc, tc, out, x, gamma)
    if compile_:
        nc.compile()
    return nc


def kernel(x: np.ndarray, gamma: np.ndarray, trace: bool = False):
    assert x.shape == (B, H, W, C), x.shape
    if "nc" not in _CACHE:
        _CACHE["nc"] = build()
    nc = _CACHE["nc"]

    g128 = np.full((128, 1), np.float32(np.asarray(gamma).reshape(-1)[0]),
                   dtype=np.float32)
    in_maps = [
        {
            "x": np.ascontiguousarray(
                np.asarray(x[i], dtype=np.float32).reshape(HW, C)),
            "gamma": g128,
        }
        for i in range(B)
    ]
    if trace:
        res = run_bass_kernel_spmd(nc, in_maps, core_ids=list(range(B)),
                                   trace=True)
    else:
        # Force-untraced: a stray BASS_TRACE in the environment would route
        # through profiling hooks this image may not have.
        import os
        prev = os.environ.get("BASS_NEVER_TRACE")
        os.environ["BASS_NEVER_TRACE"] = "1"
        try:
            res = run_bass_kernel_spmd(nc, in_maps, core_ids=list(range(B)))
        finally:
            if prev is None:
                os.environ.pop("BASS_NEVER_TRACE", None)
            else:
                os.environ["BASS_NEVER_TRACE"] = prev
    _CACHE["last_result"] = res
    out = np.stack([res.results[i]["out"] for i in range(B)], axis=0)
    return out.reshape(B, H, W, C).astype(np.float32)
